# revision 30
# baseline (speedup 1.0000x reference)
"""Trainium2 Bass kernel for multi-head attention (B=4, N=2048, C=512, 8 heads).

Sharding: 8 cores = (batch b = core//2) x (head-group g = core%2, 4 heads each).
Per core, a transposed-scores attention pipeline:
  - host supplies x[b] transposed (xT [C, N]) and per-group transposed weights,
    all pre-cast to fp16
  - qT/kT stored zero-padded per head ([:, hh, :] has head hh's 64 dims on
    its own partition range, rest zero) so score matmuls contract over the
    full K=128 partition range: same N cycles as K=64, but the PE activity
    monitor sees a fully-active array and keeps the 2.4 GHz clock (K=64
    matmuls -- even concurrent row-tile pairs -- measure at the 1.2 GHz
    throttled rate)
  - v as [N, (1+64) per head] tiles; the leading ones column makes attn@v
    emit the softmax denominator into PSUM partition 0
  - the ACT exp stream (128 x [128,1024] exps) paces the kernel; per block
    the PE does 2 score + 2 attn@v matmuls (attnv trails by 3 blocks so
    filler stalls never delay the next exp's scores)
  - PSUM: scores double-buffered (4 banks), one attn@v accumulator (2
    banks; the next section's attnv start rides on the 3-block lag while
    the norm drains), and a dedicated 2-buf pool for filler projections so
    they never steal the score rotation
  - DMA order: wk, wq, then xT[t, 0:512] quarters, so the first projection
    chunk starts as soon as possible; sections run qh-major so the output
    projection interleaves with the qh=1 sections
  - normalization off the PE: DVE fast-reciprocal, GpSimd partition
    broadcast, DVE multiply, partition-shift DMA into outT on the GpSimd
    DMA queue (the Sync queue is busy streaming y to HBM)
  - a few junk matmuls keep the PE's HAM clock warm across the final norm
    chain so the tail y blocks run at 2.4 GHz
  - host sums the two half-head partials
"""

import sys

sys.path.insert(0, "/opt/trn_rl_repo")

import numpy as np

B, N, C = 4, 2048, 512
H, D = 8, 64
SCALE = float(D) ** -0.5  # 0.125, exact in fp32
P = 128
CT = C // P  # 4 contraction tiles over channels
NT = N // P  # 16 token blocks
NCORES = 8
FD = 1024  # softmax block free dim (q chunk)
QH = N // FD  # 2 q halves
LAG = 7  # attnv trails scores by this many blocks (crosses section bounds)

_cache = {}


def _build():
    import concourse.bacc as bacc
    import concourse.tile as tile
    from concourse import mybir

    f32 = mybir.dt.float32
    f16 = mybir.dt.float16
    u16 = mybir.dt.uint16
    EXP = mybir.ActivationFunctionType.Exp

    nc = bacc.Bacc("TRN2", target_bir_lowering=False, debug=False,
                   num_devices=NCORES)

    xT_d = nc.dram_tensor("xT", [C, N], f16, kind="ExternalInput")
    wqT_d = nc.dram_tensor("wqT", [P, CT * 256], f16, kind="ExternalInput")
    wkT_d = nc.dram_tensor("wkT", [P, CT * 256], f16, kind="ExternalInput")
    wvT_d = nc.dram_tensor("wvT", [P, CT * 256], f16, kind="ExternalInput")
    pwT_d = nc.dram_tensor("pwT", [P, 2 * C], f16, kind="ExternalInput")
    y_d = nc.dram_tensor("y", [N, C], f32, kind="ExternalOutput")

    with tile.TileContext(nc) as tc:
        with (
            tc.tile_pool(name="io", bufs=1) as io,
            tc.tile_pool(name="qk", bufs=1) as qk,
            tc.tile_pool(name="expp", bufs=LAG + 2) as expp,
            tc.tile_pool(name="workp", bufs=3) as workp,
            tc.tile_pool(name="yp", bufs=4) as yp,
            tc.tile_pool(name="ps_s", bufs=2, space="PSUM") as ps_s,
            tc.tile_pool(name="ps_f", bufs=2, space="PSUM") as ps_f,
            tc.tile_pool(name="ps_o", bufs=1, space="PSUM") as ps_o,
        ):
            # ---- input loads; first-needed pieces first ----
            xT_sb = io.tile([P, CT, N], f16, tag="xT", name="xT_sb")
            xT_ap = xT_d[:].rearrange("(t p) n -> p t n", p=P)

            wk_sb = io.tile([P, CT, 256], f16, tag="wk", name="wk_sb")
            nc.sync.dma_start(
                wk_sb[:], wkT_d[:].rearrange("p (t m) -> p t m", t=CT))
            wq_sb = io.tile([P, CT, 256], f16, tag="wq", name="wq_sb")
            nc.sync.dma_start(
                wq_sb[:], wqT_d[:].rearrange("p (t m) -> p t m", t=CT))
            for t in range(CT):
                nc.sync.dma_start(xT_sb[:, t, 0:512], xT_ap[:, t, 0:512])
            wv_sb = io.tile([P, CT, 256], f16, tag="wv", name="wv_sb")
            nc.sync.dma_start(
                wv_sb[:], wvT_d[:].rearrange("p (t m) -> p t m", t=CT))
            for t in range(CT):
                nc.sync.dma_start(xT_sb[:, t, 512:1024], xT_ap[:, t, 512:1024])
            for t in range(CT):
                nc.sync.dma_start(xT_sb[:, t, 1024:2048],
                                  xT_ap[:, t, 1024:2048])
            pw_sb = io.tile([P, 2, C], f16, tag="pw", name="pw_sb")
            nc.sync.dma_start(
                pw_sb[:], pwT_d[:].rearrange("p (t m) -> p t m", t=2))

            # ---- SBUF persistents ----
            qT = []
            kT = []
            vv = []
            outT = []
            for p in range(2):
                qT.append(qk.tile([P, 2, N], f16, tag=f"qT{p}", name=f"qT{p}"))
                kT.append(qk.tile([P, 2, N], f16, tag=f"kT{p}", name=f"kT{p}"))
                vv.append(qk.tile([P, NT, 130], f16, tag=f"v{p}", name=f"v{p}"))
                outT.append(qk.tile([P, N], f16, tag=f"outT{p}", name=f"outT{p}"))

            # trigger the ACT exp table load during the DMA ramp
            scratch1 = io.tile([1, 2], f32, tag="scratch1", name="scratch1")
            nc.vector.memset(scratch1[:], 0.0)
            nc.scalar.activation(scratch1[0:1, 0:1], scratch1[0:1, 1:2], EXP)
            # zero-pads: only the hh=0 halves of pair 0 gate the first
            # scores -> DVE (fast-ish); the rest are needed >=1 section
            # later -> GpSimd (slow but otherwise idle early)
            nc.vector.memset(kT[0][64:128, 0, :], 0.0)
            nc.vector.memset(qT[0][64:128, 0, :], 0.0)
            nc.gpsimd.memset(kT[0][0:64, 1, :], 0.0)
            nc.gpsimd.memset(qT[0][0:64, 1, :], 0.0)
            nc.gpsimd.memset(kT[1][64:128, 0, :], 0.0)
            nc.gpsimd.memset(kT[1][0:64, 1, :], 0.0)
            nc.gpsimd.memset(qT[1][64:128, 0, :], 0.0)
            nc.gpsimd.memset(qT[1][0:64, 1, :], 0.0)
            for p in range(2):
                # ones columns (fp16 1.0) at the head of each v block
                nc.vector.memset(vv[p][:, :, 0:1].bitcast(u16), 0x3C00)
                nc.vector.memset(vv[p][:, :, 65:66].bitcast(u16), 0x3C00)

            def emit_qk_chunk(p, w_sb, dst, ch, dve_only=False):
                pc = slice(128 * p, 128 * (p + 1))
                cs = slice(512 * ch, 512 * (ch + 1))
                ps = ps_f.tile([P, 512], f32, tag="f",
                               name=f"qkps_{p}_{ch}_{w_sb.tensor.name}")
                for t in range(CT):
                    nc.tensor.matmul(
                        ps[:],
                        lhsT=w_sb[:, t, pc],
                        rhs=xT_sb[:, t, cs],
                        start=(t == 0), stop=(t == CT - 1))
                nc.vector.tensor_copy(dst[0:64, 0, cs], ps[0:64, :])
                if dve_only:
                    nc.vector.tensor_copy(dst[64:128, 1, cs], ps[64:128, :])
                else:
                    nc.scalar.copy(dst[64:128, 1, cs], ps[64:128, :])

            # a qk chunk split across two consecutive filler slots (2+2
            # matmuls) so a filler block's PE work stays under the exp
            # cadence; the accumulation group stays open across the split
            _half_chunks = {}

            def emit_qk_half(p, w_sb, dst, ch, second):
                pc = slice(128 * p, 128 * (p + 1))
                cs = slice(512 * ch, 512 * (ch + 1))
                key = (id(w_sb), p, ch)
                if not second:
                    ps = ps_f.tile([P, 512], f32, tag="f",
                                   name=f"qkh_{p}_{ch}_{w_sb.tensor.name}")
                    _half_chunks[key] = ps
                    for t in range(2):
                        nc.tensor.matmul(
                            ps[:], lhsT=w_sb[:, t, pc], rhs=xT_sb[:, t, cs],
                            start=(t == 0), stop=False)
                else:
                    ps = _half_chunks.pop(key)
                    for t in range(2, CT):
                        nc.tensor.matmul(
                            ps[:], lhsT=w_sb[:, t, pc], rhs=xT_sb[:, t, cs],
                            start=False, stop=(t == CT - 1))
                    nc.vector.tensor_copy(dst[0:64, 0, cs], ps[0:64, :])
                    nc.vector.tensor_copy(dst[64:128, 1, cs], ps[64:128, :])

            def emit_v_tile(tt):
                psv = ps_f.tile([P, 512], f32, tag="f", name=f"vps_{tt}")
                for t in range(CT):
                    nc.tensor.matmul(
                        psv[:, 0:256],
                        lhsT=xT_sb[:, t, 128 * tt:128 * (tt + 1)],
                        rhs=wv_sb[:, t, 0:256],
                        start=(t == 0), stop=(t == CT - 1))
                for p in range(2):
                    pv = psv[:, 128 * p:128 * (p + 1)].rearrange(
                        "p (two d) -> p two d", two=2)
                    dv = vv[p][:, tt, 0:130].rearrange(
                        "p (two d65) -> p two d65", two=2)[:, :, 1:65]
                    nc.vector.tensor_copy(dv, pv)

            def emit_y_block(tt, act_evict=False, split_evict=False):
                yps = ps_f.tile([P, 512], f32, tag="f", name=f"yps_{tt}")
                for p in range(2):
                    nc.tensor.matmul(
                        yps[:], lhsT=outT[p][:, 128 * tt:128 * (tt + 1)],
                        rhs=pw_sb[:, p, :], start=(p == 0), stop=(p == 1))
                ys = yp.tile([P, C], f32, tag="y", name=f"ys_{tt}")
                if split_evict:
                    # tail blocks: ACT and DVE each evict half the free dim
                    # in parallel (both engines are idle there)
                    nc.scalar.copy(ys[:, 0:256], yps[:, 0:256])
                    nc.vector.tensor_copy(ys[:, 256:512], yps[:, 256:512])
                elif act_evict:
                    nc.scalar.copy(ys[:], yps[:])
                else:
                    nc.vector.tensor_copy(ys[:], yps[:])
                nc.sync.dma_start(y_d[128 * tt:128 * (tt + 1), :], ys[:])

            def norm_head(p, hh, qh, o):
                # evict o to SBUF first: the PSUM accumulator frees after
                # one DVE copy (~1.2us) instead of after the whole
                # recip/broadcast/mul chain (~4.5us), so the next section's
                # first attnv (sharing the single ps_o buffer) never stalls
                qs = slice(FD * qh, FD * (qh + 1))
                oc = workp.tile([65, FD], f32, tag="oc",
                                name=f"oc_{p}_{hh}_{qh}")
                nc.vector.tensor_copy(oc[:], o[:])
                r = workp.tile([P, FD], f32, tag="r", name=f"r_{p}_{hh}_{qh}")
                nc.vector.reciprocal_approx_fast(r[0:1, :], oc[0:1, :])
                rb = workp.tile([65, FD], f32, tag="rb",
                                name=f"rb_{p}_{hh}_{qh}")
                nc.gpsimd.partition_broadcast(rb[:], r[0:1, :])
                st = workp.tile([65, FD], f16, tag="st",
                                name=f"st_{p}_{hh}_{qh}")
                nc.vector.tensor_mul(st[:], oc[:], rb[:])
                nc.gpsimd.dma_start(outT[p][64 * hh:64 * (hh + 1), qs],
                                    st[1:65, :])

            def norm_head_split(p, hh, qh, o):
                # two 512-wide halves with interleaved emission so the DVE
                # and GpSimd stages of both halves pipeline: the first y
                # blocks gated on this norm start ~3us earlier
                r = workp.tile([P, FD], f32, tag="r", name=f"rs_{p}_{hh}_{qh}")
                rb = workp.tile([65, FD], f32, tag="rb",
                                name=f"rbs_{p}_{hh}_{qh}")
                st = workp.tile([65, FD], f16, tag="st",
                                name=f"sts_{p}_{hh}_{qh}")
                for half in range(2):
                    fs = slice(512 * half, 512 * (half + 1))
                    nc.vector.reciprocal_approx_fast(r[0:1, fs], o[0:1, fs])
                for half in range(2):
                    fs = slice(512 * half, 512 * (half + 1))
                    nc.gpsimd.partition_broadcast(rb[:, fs], r[0:1, fs])
                for half in range(2):
                    fs = slice(512 * half, 512 * (half + 1))
                    qs = slice(FD * qh + 512 * half, FD * qh + 512 * (half + 1))
                    nc.vector.tensor_mul(st[:, fs], o[:, fs], rb[:, fs])
                    nc.gpsimd.dma_start(outT[p][64 * hh:64 * (hh + 1), qs],
                                        st[1:65, fs])

            # ---- critical prefix ----
            emit_qk_chunk(0, wk_sb, kT[0], 0)
            emit_qk_chunk(0, wq_sb, qT[0], 0)
            emit_qk_chunk(0, wq_sb, qT[0], 1)
            emit_v_tile(0)
            emit_v_tile(1)
            emit_v_tile(2)

            def f_v(tt):
                return lambda: emit_v_tile(tt)

            def f_k(p, ch):
                return lambda: emit_qk_chunk(p, wk_sb, kT[p], ch,
                                             dve_only=True)

            def f_ka(p, ch):
                return lambda: emit_qk_half(p, wk_sb, kT[p], ch, False)

            def f_kb(p, ch):
                return lambda: emit_qk_half(p, wk_sb, kT[p], ch, True)

            def f_qa(p, ch):
                return lambda: emit_qk_half(p, wq_sb, qT[p], ch, False)

            def f_qb(p, ch):
                return lambda: emit_qk_half(p, wq_sb, qT[p], ch, True)

            def f_y(tt, act_evict=False):
                return lambda: emit_y_block(tt, act_evict)

            def plan(assignments):
                out = [[] for _ in range(NT + 1)]
                for i, ths in assignments.items():
                    out[i] = ths
                return out

            # deps: scores(blk i) needs kT chunk i//4; attnv(i) (emitted at
            # block i+LAG) needs v tile i; section 2 needs kT[1]+qT[1]
            # chunks 0,1; sections 4+ need q chunks 2,3; y blocks 0..7 need
            # every qh=0 norm (done after section 3).
            sec_fillers = {
                0: plan({0: [f_v(3)], 1: [f_k(0, 1)], 2: [f_v(4)],
                         3: [f_v(5)], 4: [f_v(6)], 5: [f_k(0, 2)],
                         6: [f_v(7)], 7: [f_v(8)], 8: [f_v(9)],
                         9: [f_k(0, 3)], 10: [f_v(10)], 11: [f_v(11)],
                         12: [f_v(12)], 13: [f_v(13)], 14: [f_v(14)],
                         15: [f_v(15)]}),
                1: plan({1: [f_ka(1, 0)], 2: [f_kb(1, 0)],
                         4: [f_ka(1, 1)], 5: [f_kb(1, 1)],
                         7: [f_ka(1, 2)], 8: [f_kb(1, 2)],
                         10: [f_ka(1, 3)], 11: [f_kb(1, 3)],
                         12: [f_qa(1, 0)], 13: [f_qb(1, 0)],
                         14: [f_qa(1, 1)], 15: [f_qb(1, 1)]}),
                2: plan({2: [f_qa(0, 2)], 3: [f_qb(0, 2)],
                         6: [f_qa(0, 3)], 7: [f_qb(0, 3)]}),
                3: plan({2: [f_qa(1, 2)], 3: [f_qb(1, 2)],
                         6: [f_qa(1, 3)], 7: [f_qb(1, 3)]}),
                # norm(sec3) is emitted at sec4 block LAG-1 and its outT
                # write lands ~6us later: y fillers must come after it in
                # emission order (dep visibility) AND late enough that the
                # outT DMA has landed (no FIFO stall)
                4: plan({12: [f_y(0)], 14: [f_y(1)]}),
                5: plan({2: [f_y(2)], 4: [f_y(3)], 6: [f_y(4)],
                         8: [f_y(5)], 10: [f_y(6)], 12: [f_y(7)]}),
                6: plan({}),
                7: plan({}),
            }

            sections = [(0, 0, 0), (0, 1, 0), (1, 0, 0), (1, 1, 0),
                        (0, 0, 1), (0, 1, 1), (1, 0, 1), (1, 1, 1)]
            LAST = len(sections) - 1

            # ---- one global pipelined stream: scores+exp lead, attn@v
            # trails LAG blocks behind and flows straight across section
            # boundaries (no drain between sections, so a trailing attnv
            # waiting on its exp never blocks the next section's scores).
            # Each section's norm is emitted when its last attnv pops; the
            # single attn@v accumulator (ps_o bufs=1) is re-allocated at the
            # pop of a section's first attnv, by which time the previous
            # norm's reads have drained. ----
            o_tiles = {}
            pending = []

            def emit_scores_exp(idx, p, hh, qh, i):
                ks = slice(128 * i, 128 * (i + 1))
                s = ps_s.tile([P, FD], f32, tag="s", name=f"s_{idx}_{i}")
                for j in range(2):
                    js = slice(512 * j, 512 * (j + 1))
                    qj = slice(FD * qh + 512 * j, FD * qh + 512 * (j + 1))
                    nc.tensor.matmul(
                        s[:, js], lhsT=kT[p][:, hh, ks],
                        rhs=qT[p][:, hh, qj], start=True, stop=True)
                e = expp.tile([P, FD], f16, tag="exp", name=f"e_{idx}_{i}")
                nc.scalar.activation(e[:], s[:], EXP)
                return e

            last_e = [None]

            def pop_attnv():
                idx, p, hh, qh, i, e = pending.pop(0)
                last_e[0] = e
                if i == 0:
                    o_tiles[idx] = ps_o.tile([65, FD], f32, tag="o",
                                             name=f"o_{idx}")
                o = o_tiles[idx]
                vs = slice(65 * hh, 65 * (hh + 1))
                for j in range(2):
                    js = slice(512 * j, 512 * (j + 1))
                    nc.tensor.matmul(
                        o[:, js], lhsT=vv[p][:, i, vs], rhs=e[:, js],
                        start=(i == 0), stop=(i == NT - 1))
                if i == NT - 1:
                    if idx == LAST:
                        norm_head_split(p, hh, qh, o)
                    else:
                        norm_head(p, hh, qh, o)

            for idx, (p, hh, qh) in enumerate(sections):
                fillers = sec_fillers[idx]
                for i in range(NT):
                    pending.append((idx, p, hh, qh, i,
                                    emit_scores_exp(idx, p, hh, qh, i)))
                    for f in fillers[i]:
                        f()
                    if len(pending) > LAG:
                        pop_attnv()
            while pending:
                pop_attnv()

            # ---- tail: junk matmuls bridge the PE across the final norm
            # chain so HAM keeps 2.4 GHz for the y blocks.  Their rhs is the
            # LAST exp's output tile, which pins them after the end of the
            # exp stream (a dep-free matmul gets floated to an arbitrary
            # slot by the scheduler), then the y blocks for the second
            # token half ----
            ps_w = ps_f.tile([P, 512], f32, tag="f", name="warm")
            for i in range(12):
                nc.tensor.matmul(ps_w[:], lhsT=pw_sb[:, 0, 0:128],
                                 rhs=last_e[0][:, 0:512], start=(i == 0),
                                 stop=(i == 11))
            nc.vector.tensor_copy(scratch1[0:1, 0:2], ps_w[0:1, 0:2])
            for tt in range(8, NT):
                emit_y_block(tt, split_evict=True)

    nc.finalize()
    return nc


def _get_nc():
    if "nc" not in _cache:
        _cache["nc"] = _build()
    return _cache["nc"]


def _pack(wt, groups):
    # [G*128, M] row-major -> [128, G*M]: partition p holds the concat over
    # groups of row (g*128 + p), so the DMA reads one contiguous run per p
    g128, m = wt.shape
    assert g128 == groups * 128
    return np.ascontiguousarray(
        wt.reshape(groups, 128, m).transpose(1, 0, 2).reshape(128, groups * m))


def _make_in_maps(x, q_w, kv_w, proj_w):
    x = np.asarray(x, dtype=np.float32)
    q_w = np.asarray(q_w, dtype=np.float32)
    kv_w = np.asarray(kv_w, dtype=np.float32)
    proj_w = np.asarray(proj_w, dtype=np.float32)
    f16 = np.float16
    in_maps = []
    for core in range(NCORES):
        b, g = core // 2, core % 2
        hs = slice(g * 256, (g + 1) * 256)
        in_maps.append({
            "xT": np.ascontiguousarray(x[b].T.astype(f16)),
            "wqT": _pack((q_w[hs, :] * np.float32(SCALE)).T.astype(f16), CT),
            "wkT": _pack(kv_w[hs, :].T.astype(f16), CT),
            "wvT": _pack(
                kv_w[C + g * 256:C + (g + 1) * 256, :].T.astype(f16), CT),
            "pwT": _pack(proj_w[:, hs].T.astype(f16), 2),
        })
    return in_maps


def kernel(x, q_w, kv_w, proj_w, proj_b, H=None, W=None, _trace=False):
    from concourse.bass_utils import run_bass_kernel_spmd

    nc = _get_nc()
    in_maps = _make_in_maps(x, q_w, kv_w, proj_w)
    res = run_bass_kernel_spmd(nc, in_maps, core_ids=list(range(NCORES)),
                               trace=_trace)
    proj_b = np.asarray(proj_b, dtype=np.float32)
    out = np.empty((B, N, C), dtype=np.float32)
    for b in range(B):
        out[b] = res.results[2 * b]["y"] + res.results[2 * b + 1]["y"] + proj_b
    if _trace:
        return out, res
    return out


# revision 33
# speedup vs baseline: 1.0105x; 1.0105x over previous
"""Trainium2 Bass kernel for multi-head attention (B=4, N=2048, C=512, 8 heads).

Sharding: 8 cores = (batch b = core//2) x (head-group g = core%2, 4 heads each).
Per core, a transposed-scores attention pipeline:
  - host supplies x[b] transposed (xT [C, N]) and per-group transposed weights,
    all pre-cast to fp16
  - qT/kT stored zero-padded per head ([:, hh, :] has head hh's 64 dims on
    its own partition range, rest zero) so score matmuls contract over the
    full K=128 partition range: same N cycles as K=64, but the PE activity
    monitor sees a fully-active array and keeps the 2.4 GHz clock (K=64
    matmuls -- even concurrent row-tile pairs -- measure at the 1.2 GHz
    throttled rate)
  - v as [N, (1+64) per head] tiles; the leading ones column makes attn@v
    emit the softmax denominator into PSUM partition 0
  - the ACT exp stream (128 x [128,1024] exps) paces the kernel; per block
    the PE does 2 score + 2 attn@v matmuls (attnv trails by 3 blocks so
    filler stalls never delay the next exp's scores)
  - PSUM: scores double-buffered (4 banks), one attn@v accumulator (2
    banks; the next section's attnv start rides on the 3-block lag while
    the norm drains), and a dedicated 2-buf pool for filler projections so
    they never steal the score rotation
  - DMA order: wk, wq, then xT[t, 0:512] quarters, so the first projection
    chunk starts as soon as possible; sections run qh-major so the output
    projection interleaves with the qh=1 sections
  - normalization off the PE: DVE fast-reciprocal, GpSimd partition
    broadcast, DVE multiply, partition-shift DMA into outT on the GpSimd
    DMA queue (the Sync queue is busy streaming y to HBM)
  - a few junk matmuls keep the PE's HAM clock warm across the final norm
    chain so the tail y blocks run at 2.4 GHz
  - host sums the two half-head partials
"""

import sys

sys.path.insert(0, "/opt/trn_rl_repo")

import numpy as np

B, N, C = 4, 2048, 512
H, D = 8, 64
SCALE = float(D) ** -0.5  # 0.125, exact in fp32
P = 128
CT = C // P  # 4 contraction tiles over channels
NT = N // P  # 16 token blocks
NCORES = 8
FD = 1024  # softmax block free dim (q chunk)
QH = N // FD  # 2 q halves
LAG = 7  # attnv trails scores by this many blocks (crosses section bounds)

_cache = {}


def _build():
    import concourse.bacc as bacc
    import concourse.tile as tile
    from concourse import mybir

    f32 = mybir.dt.float32
    f16 = mybir.dt.float16
    u16 = mybir.dt.uint16
    EXP = mybir.ActivationFunctionType.Exp

    nc = bacc.Bacc("TRN2", target_bir_lowering=False, debug=False,
                   num_devices=NCORES)

    xT_d = nc.dram_tensor("xT", [C, N], f16, kind="ExternalInput")
    wqT_d = nc.dram_tensor("wqT", [P, CT * 256], f16, kind="ExternalInput")
    wkT_d = nc.dram_tensor("wkT", [P, CT * 256], f16, kind="ExternalInput")
    wvT_d = nc.dram_tensor("wvT", [P, CT * 256], f16, kind="ExternalInput")
    pwT_d = nc.dram_tensor("pwT", [P, 2 * C], f16, kind="ExternalInput")
    y_d = nc.dram_tensor("y", [N, C], f32, kind="ExternalOutput")

    with tile.TileContext(nc) as tc:
        with (
            tc.tile_pool(name="io", bufs=1) as io,
            tc.tile_pool(name="qk", bufs=1) as qk,
            tc.tile_pool(name="expp", bufs=LAG + 2) as expp,
            tc.tile_pool(name="workp", bufs=3) as workp,
            tc.tile_pool(name="yp", bufs=4) as yp,
            tc.tile_pool(name="ps_s", bufs=2, space="PSUM") as ps_s,
            tc.tile_pool(name="ps_f", bufs=2, space="PSUM") as ps_f,
            tc.tile_pool(name="ps_o", bufs=1, space="PSUM") as ps_o,
        ):
            # ---- input loads; first-needed pieces first ----
            xT_sb = io.tile([P, CT, N], f16, tag="xT", name="xT_sb")
            xT_ap = xT_d[:].rearrange("(t p) n -> p t n", p=P)

            wk_sb = io.tile([P, CT, 256], f16, tag="wk", name="wk_sb")
            nc.sync.dma_start(
                wk_sb[:], wkT_d[:].rearrange("p (t m) -> p t m", t=CT))
            wq_sb = io.tile([P, CT, 256], f16, tag="wq", name="wq_sb")
            nc.sync.dma_start(
                wq_sb[:], wqT_d[:].rearrange("p (t m) -> p t m", t=CT))
            for t in range(CT):
                nc.sync.dma_start(xT_sb[:, t, 0:512], xT_ap[:, t, 0:512])
            wv_sb = io.tile([P, CT, 256], f16, tag="wv", name="wv_sb")
            nc.sync.dma_start(
                wv_sb[:], wvT_d[:].rearrange("p (t m) -> p t m", t=CT))
            for t in range(CT):
                nc.sync.dma_start(xT_sb[:, t, 512:1024], xT_ap[:, t, 512:1024])
            for t in range(CT):
                nc.sync.dma_start(xT_sb[:, t, 1024:2048],
                                  xT_ap[:, t, 1024:2048])
            pw_sb = io.tile([P, 2, C], f16, tag="pw", name="pw_sb")
            nc.sync.dma_start(
                pw_sb[:], pwT_d[:].rearrange("p (t m) -> p t m", t=2))

            # ---- SBUF persistents ----
            qT = []
            kT = []
            vv = []
            outT = []
            for p in range(2):
                qT.append(qk.tile([P, 2, N], f16, tag=f"qT{p}", name=f"qT{p}"))
                kT.append(qk.tile([P, 2, N], f16, tag=f"kT{p}", name=f"kT{p}"))
                vv.append(qk.tile([P, NT, 130], f16, tag=f"v{p}", name=f"v{p}"))
                outT.append(qk.tile([P, N], f16, tag=f"outT{p}", name=f"outT{p}"))

            # trigger the ACT exp table load during the DMA ramp
            scratch1 = io.tile([1, 2], f32, tag="scratch1", name="scratch1")
            nc.vector.memset(scratch1[:], 0.0)
            nc.scalar.activation(scratch1[0:1, 0:1], scratch1[0:1, 1:2], EXP)
            # zero-pads: only the hh=0 halves of pair 0 gate the first
            # scores -> DVE (fast-ish); the rest are needed >=1 section
            # later -> GpSimd (slow but otherwise idle early)
            nc.vector.memset(kT[0][64:128, 0, :], 0.0)
            nc.vector.memset(qT[0][64:128, 0, :], 0.0)
            nc.gpsimd.memset(kT[0][0:64, 1, :], 0.0)
            nc.gpsimd.memset(qT[0][0:64, 1, :], 0.0)
            nc.gpsimd.memset(kT[1][64:128, 0, :], 0.0)
            nc.gpsimd.memset(kT[1][0:64, 1, :], 0.0)
            nc.gpsimd.memset(qT[1][64:128, 0, :], 0.0)
            nc.gpsimd.memset(qT[1][0:64, 1, :], 0.0)
            for p in range(2):
                # ones columns (fp16 1.0) at the head of each v block
                nc.vector.memset(vv[p][:, :, 0:1].bitcast(u16), 0x3C00)
                nc.vector.memset(vv[p][:, :, 65:66].bitcast(u16), 0x3C00)

            def emit_qk_chunk(p, w_sb, dst, ch, dve_only=False,
                              split_first=False):
                pc = slice(128 * p, 128 * (p + 1))
                cs = slice(512 * ch, 512 * (ch + 1))
                ps = ps_f.tile([P, 512], f32, tag="f",
                               name=f"qkps_{p}_{ch}_{w_sb.tensor.name}")
                for t in range(CT):
                    nc.tensor.matmul(
                        ps[:],
                        lhsT=w_sb[:, t, pc],
                        rhs=xT_sb[:, t, cs],
                        start=(t == 0), stop=(t == CT - 1))
                if split_first:
                    # land the first 128 tokens first: the first scores
                    # matmul only reads those, so it unblocks ~0.4us sooner
                    nc.vector.tensor_copy(dst[0:64, 0, 512 * ch:512 * ch + 128],
                                          ps[0:64, 0:128])
                    nc.vector.tensor_copy(
                        dst[0:64, 0, 512 * ch + 128:512 * (ch + 1)],
                        ps[0:64, 128:512])
                else:
                    nc.vector.tensor_copy(dst[0:64, 0, cs], ps[0:64, :])
                if dve_only:
                    nc.vector.tensor_copy(dst[64:128, 1, cs], ps[64:128, :])
                else:
                    nc.scalar.copy(dst[64:128, 1, cs], ps[64:128, :])

            # a qk chunk split across two consecutive filler slots (2+2
            # matmuls) so a filler block's PE work stays under the exp
            # cadence; the accumulation group stays open across the split
            _half_chunks = {}

            def emit_qk_half(p, w_sb, dst, ch, second):
                pc = slice(128 * p, 128 * (p + 1))
                cs = slice(512 * ch, 512 * (ch + 1))
                key = (id(w_sb), p, ch)
                if not second:
                    ps = ps_f.tile([P, 512], f32, tag="f",
                                   name=f"qkh_{p}_{ch}_{w_sb.tensor.name}")
                    _half_chunks[key] = ps
                    for t in range(2):
                        nc.tensor.matmul(
                            ps[:], lhsT=w_sb[:, t, pc], rhs=xT_sb[:, t, cs],
                            start=(t == 0), stop=False)
                else:
                    ps = _half_chunks.pop(key)
                    for t in range(2, CT):
                        nc.tensor.matmul(
                            ps[:], lhsT=w_sb[:, t, pc], rhs=xT_sb[:, t, cs],
                            start=False, stop=(t == CT - 1))
                    nc.vector.tensor_copy(dst[0:64, 0, cs], ps[0:64, :])
                    nc.vector.tensor_copy(dst[64:128, 1, cs], ps[64:128, :])

            def emit_v_tile(tt):
                psv = ps_f.tile([P, 512], f32, tag="f", name=f"vps_{tt}")
                for t in range(CT):
                    nc.tensor.matmul(
                        psv[:, 0:256],
                        lhsT=xT_sb[:, t, 128 * tt:128 * (tt + 1)],
                        rhs=wv_sb[:, t, 0:256],
                        start=(t == 0), stop=(t == CT - 1))
                for p in range(2):
                    pv = psv[:, 128 * p:128 * (p + 1)].rearrange(
                        "p (two d) -> p two d", two=2)
                    dv = vv[p][:, tt, 0:130].rearrange(
                        "p (two d65) -> p two d65", two=2)[:, :, 1:65]
                    nc.vector.tensor_copy(dv, pv)

            def emit_y_block(tt, act_evict=False, split_evict=False):
                yps = ps_f.tile([P, 512], f32, tag="f", name=f"yps_{tt}")
                for p in range(2):
                    nc.tensor.matmul(
                        yps[:], lhsT=outT[p][:, 128 * tt:128 * (tt + 1)],
                        rhs=pw_sb[:, p, :], start=(p == 0), stop=(p == 1))
                ys = yp.tile([P, C], f32, tag="y", name=f"ys_{tt}")
                if split_evict:
                    # tail blocks: ACT and DVE each evict half the free dim
                    # in parallel (both engines are idle there)
                    nc.scalar.copy(ys[:, 0:256], yps[:, 0:256])
                    nc.vector.tensor_copy(ys[:, 256:512], yps[:, 256:512])
                elif act_evict:
                    nc.scalar.copy(ys[:], yps[:])
                else:
                    nc.vector.tensor_copy(ys[:], yps[:])
                nc.sync.dma_start(y_d[128 * tt:128 * (tt + 1), :], ys[:])

            def norm_head(p, hh, qh, o):
                # evict o to SBUF first: the PSUM accumulator frees after
                # one DVE copy (~1.2us) instead of after the whole
                # recip/broadcast/mul chain (~4.5us), so the next section's
                # first attnv (sharing the single ps_o buffer) never stalls
                qs = slice(FD * qh, FD * (qh + 1))
                oc = workp.tile([65, FD], f32, tag="oc",
                                name=f"oc_{p}_{hh}_{qh}")
                nc.vector.tensor_copy(oc[:], o[:])
                r = workp.tile([P, FD], f32, tag="r", name=f"r_{p}_{hh}_{qh}")
                nc.vector.reciprocal_approx_fast(r[0:1, :], oc[0:1, :])
                rb = workp.tile([65, FD], f32, tag="rb",
                                name=f"rb_{p}_{hh}_{qh}")
                nc.gpsimd.partition_broadcast(rb[:], r[0:1, :])
                st = workp.tile([65, FD], f16, tag="st",
                                name=f"st_{p}_{hh}_{qh}")
                nc.vector.tensor_mul(st[:], oc[:], rb[:])
                nc.gpsimd.dma_start(outT[p][64 * hh:64 * (hh + 1), qs],
                                    st[1:65, :])

            def norm_head_split(p, hh, qh, o):
                # two 512-wide halves with interleaved emission so the DVE
                # and GpSimd stages of both halves pipeline: the first y
                # blocks gated on this norm start ~3us earlier
                r = workp.tile([P, FD], f32, tag="r", name=f"rs_{p}_{hh}_{qh}")
                rb = workp.tile([65, FD], f32, tag="rb",
                                name=f"rbs_{p}_{hh}_{qh}")
                st = workp.tile([65, FD], f16, tag="st",
                                name=f"sts_{p}_{hh}_{qh}")
                for half in range(2):
                    fs = slice(512 * half, 512 * (half + 1))
                    nc.vector.reciprocal_approx_fast(r[0:1, fs], o[0:1, fs])
                for half in range(2):
                    fs = slice(512 * half, 512 * (half + 1))
                    nc.gpsimd.partition_broadcast(rb[:, fs], r[0:1, fs])
                for half in range(2):
                    fs = slice(512 * half, 512 * (half + 1))
                    qs = slice(FD * qh + 512 * half, FD * qh + 512 * (half + 1))
                    nc.vector.tensor_mul(st[:, fs], o[:, fs], rb[:, fs])
                    nc.gpsimd.dma_start(outT[p][64 * hh:64 * (hh + 1), qs],
                                        st[1:65, fs])

            # ---- critical prefix ----
            emit_qk_chunk(0, wk_sb, kT[0], 0, split_first=True)
            emit_qk_chunk(0, wq_sb, qT[0], 0)
            emit_qk_chunk(0, wq_sb, qT[0], 1)
            emit_v_tile(0)
            emit_v_tile(1)
            emit_v_tile(2)

            def f_v(tt):
                return lambda: emit_v_tile(tt)

            def f_k(p, ch):
                return lambda: emit_qk_chunk(p, wk_sb, kT[p], ch,
                                             dve_only=True)

            def f_ka(p, ch):
                return lambda: emit_qk_half(p, wk_sb, kT[p], ch, False)

            def f_kb(p, ch):
                return lambda: emit_qk_half(p, wk_sb, kT[p], ch, True)

            def f_qa(p, ch):
                return lambda: emit_qk_half(p, wq_sb, qT[p], ch, False)

            def f_qb(p, ch):
                return lambda: emit_qk_half(p, wq_sb, qT[p], ch, True)

            def f_y(tt, act_evict=False):
                return lambda: emit_y_block(tt, act_evict)

            def plan(assignments):
                out = [[] for _ in range(NT + 1)]
                for i, ths in assignments.items():
                    out[i] = ths
                return out

            # deps: scores(blk i) needs kT chunk i//4; attnv(i) (emitted at
            # block i+LAG) needs v tile i; section 2 needs kT[1]+qT[1]
            # chunks 0,1; sections 4+ need q chunks 2,3; y blocks 0..7 need
            # every qh=0 norm (done after section 3).
            sec_fillers = {
                0: plan({0: [f_v(3)], 1: [f_k(0, 1)], 2: [f_v(4)],
                         3: [f_v(5)], 4: [f_v(6)], 5: [f_k(0, 2)],
                         6: [f_v(7)], 7: [f_v(8)], 8: [f_v(9)],
                         9: [f_k(0, 3)], 10: [f_v(10)], 11: [f_v(11)],
                         12: [f_v(12)], 13: [f_v(13)], 14: [f_v(14)],
                         15: [f_v(15)]}),
                1: plan({1: [f_ka(1, 0)], 2: [f_kb(1, 0)],
                         4: [f_ka(1, 1)], 5: [f_kb(1, 1)],
                         7: [f_ka(1, 2)], 8: [f_kb(1, 2)],
                         10: [f_ka(1, 3)], 11: [f_kb(1, 3)],
                         12: [f_qa(1, 0)], 13: [f_qb(1, 0)],
                         14: [f_qa(1, 1)], 15: [f_qb(1, 1)]}),
                2: plan({2: [f_qa(0, 2)], 3: [f_qb(0, 2)],
                         6: [f_qa(0, 3)], 7: [f_qb(0, 3)]}),
                3: plan({2: [f_qa(1, 2)], 3: [f_qb(1, 2)],
                         6: [f_qa(1, 3)], 7: [f_qb(1, 3)]}),
                # norm(sec3) is emitted at sec4 block LAG-1 and its outT
                # write lands ~6us later: y fillers must come after it in
                # emission order (dep visibility) AND late enough that the
                # outT DMA has landed (no FIFO stall)
                4: plan({12: [f_y(0)], 14: [f_y(1)]}),
                5: plan({2: [f_y(2)], 4: [f_y(3)], 6: [f_y(4)],
                         8: [f_y(5)], 10: [f_y(6)], 12: [f_y(7)]}),
                6: plan({}),
                7: plan({}),
            }

            sections = [(0, 0, 0), (0, 1, 0), (1, 0, 0), (1, 1, 0),
                        (0, 0, 1), (0, 1, 1), (1, 0, 1), (1, 1, 1)]
            LAST = len(sections) - 1

            # ---- one global pipelined stream: scores+exp lead, attn@v
            # trails LAG blocks behind and flows straight across section
            # boundaries (no drain between sections, so a trailing attnv
            # waiting on its exp never blocks the next section's scores).
            # Each section's norm is emitted when its last attnv pops; the
            # single attn@v accumulator (ps_o bufs=1) is re-allocated at the
            # pop of a section's first attnv, by which time the previous
            # norm's reads have drained. ----
            o_tiles = {}
            pending = []

            def emit_scores_exp(idx, p, hh, qh, i):
                ks = slice(128 * i, 128 * (i + 1))
                s = ps_s.tile([P, FD], f32, tag="s", name=f"s_{idx}_{i}")
                e = expp.tile([P, FD], f16, tag="exp", name=f"e_{idx}_{i}")
                if idx == 0 and i == 0:
                    # very first block: half-width exps interleaved with the
                    # js matmuls, so the ACT stream starts as soon as q
                    # chunk 0 has landed (chunk 1 is still being evicted)
                    for j in range(2):
                        js = slice(512 * j, 512 * (j + 1))
                        qj = slice(FD * qh + 512 * j, FD * qh + 512 * (j + 1))
                        nc.tensor.matmul(
                            s[:, js], lhsT=kT[p][:, hh, ks],
                            rhs=qT[p][:, hh, qj], start=True, stop=True)
                        nc.scalar.activation(e[:, js], s[:, js], EXP)
                    return e
                for j in range(2):
                    js = slice(512 * j, 512 * (j + 1))
                    qj = slice(FD * qh + 512 * j, FD * qh + 512 * (j + 1))
                    nc.tensor.matmul(
                        s[:, js], lhsT=kT[p][:, hh, ks],
                        rhs=qT[p][:, hh, qj], start=True, stop=True)
                nc.scalar.activation(e[:], s[:], EXP)
                return e

            last_e = [None]

            def pop_attnv():
                idx, p, hh, qh, i, e = pending.pop(0)
                last_e[0] = e
                if i == 0:
                    o_tiles[idx] = ps_o.tile([65, FD], f32, tag="o",
                                             name=f"o_{idx}")
                o = o_tiles[idx]
                vs = slice(65 * hh, 65 * (hh + 1))
                for j in range(2):
                    js = slice(512 * j, 512 * (j + 1))
                    nc.tensor.matmul(
                        o[:, js], lhsT=vv[p][:, i, vs], rhs=e[:, js],
                        start=(i == 0), stop=(i == NT - 1))
                if i == NT - 1:
                    if idx == LAST:
                        norm_head_split(p, hh, qh, o)
                    else:
                        norm_head(p, hh, qh, o)

            for idx, (p, hh, qh) in enumerate(sections):
                fillers = sec_fillers[idx]
                for i in range(NT):
                    pending.append((idx, p, hh, qh, i,
                                    emit_scores_exp(idx, p, hh, qh, i)))
                    for f in fillers[i]:
                        f()
                    if len(pending) > LAG:
                        pop_attnv()
            while pending:
                pop_attnv()

            # ---- tail: junk matmuls bridge the PE across the final norm
            # chain so HAM keeps 2.4 GHz for the y blocks.  Their rhs is the
            # LAST exp's output tile, which pins them after the end of the
            # exp stream (a dep-free matmul gets floated to an arbitrary
            # slot by the scheduler), then the y blocks for the second
            # token half ----
            ps_w = ps_f.tile([P, 512], f32, tag="f", name="warm")
            for i in range(12):
                nc.tensor.matmul(ps_w[:], lhsT=pw_sb[:, 0, 0:128],
                                 rhs=last_e[0][:, 0:512], start=(i == 0),
                                 stop=(i == 11))
            nc.vector.tensor_copy(scratch1[0:1, 0:2], ps_w[0:1, 0:2])
            for tt in range(8, NT):
                emit_y_block(tt, split_evict=True)

    nc.finalize()
    return nc


def _get_nc():
    if "nc" not in _cache:
        _cache["nc"] = _build()
    return _cache["nc"]


def _pack(wt, groups):
    # [G*128, M] row-major -> [128, G*M]: partition p holds the concat over
    # groups of row (g*128 + p), so the DMA reads one contiguous run per p
    g128, m = wt.shape
    assert g128 == groups * 128
    return np.ascontiguousarray(
        wt.reshape(groups, 128, m).transpose(1, 0, 2).reshape(128, groups * m))


def _make_in_maps(x, q_w, kv_w, proj_w):
    x = np.asarray(x, dtype=np.float32)
    q_w = np.asarray(q_w, dtype=np.float32)
    kv_w = np.asarray(kv_w, dtype=np.float32)
    proj_w = np.asarray(proj_w, dtype=np.float32)
    f16 = np.float16
    in_maps = []
    for core in range(NCORES):
        b, g = core // 2, core % 2
        hs = slice(g * 256, (g + 1) * 256)
        in_maps.append({
            "xT": np.ascontiguousarray(x[b].T.astype(f16)),
            "wqT": _pack((q_w[hs, :] * np.float32(SCALE)).T.astype(f16), CT),
            "wkT": _pack(kv_w[hs, :].T.astype(f16), CT),
            "wvT": _pack(
                kv_w[C + g * 256:C + (g + 1) * 256, :].T.astype(f16), CT),
            "pwT": _pack(proj_w[:, hs].T.astype(f16), 2),
        })
    return in_maps


def kernel(x, q_w, kv_w, proj_w, proj_b, H=None, W=None, _trace=False):
    from concourse.bass_utils import run_bass_kernel_spmd

    nc = _get_nc()
    in_maps = _make_in_maps(x, q_w, kv_w, proj_w)
    res = run_bass_kernel_spmd(nc, in_maps, core_ids=list(range(NCORES)),
                               trace=_trace)
    proj_b = np.asarray(proj_b, dtype=np.float32)
    out = np.empty((B, N, C), dtype=np.float32)
    for b in range(B):
        out[b] = res.results[2 * b]["y"] + res.results[2 * b + 1]["y"] + proj_b
    if _trace:
        return out, res
    return out
